# revision 10
# baseline (speedup 1.0000x reference)
"""DistanceLoss kernel for 8 Trainium2 NeuronCores.

Reference computation (T=64, H=32, W=8, B=2048):
    belongs = target.T                              # [T, B] in {0,1}
    iwd  = sum_w inner_window_distances             # [T, H, B]
    cow  = sum_w outer_window_distances             # [T, H, B]
    bl   = belongs*(1-cont)*(ofd + iwd)             # [T, H, B]
    nbl  = (1-belongs)*cont*(ifd + cow)             # [T, H, B]
    loss = mean_b sum_t [ min_h bl + max_h nbl ]

Because c1 = belongs*(1-cont) and c2 = (1-belongs)*cont are constant over h
and take values in {0,1}:  min_h bl == c1 * min_h(ofd+iwd)  and
max_h nbl == c2 * max_h(ifd+cow)  exactly.

Sharding: T is split 8 ways (8 towns per core); per-core slabs of the two
big [T,H,W,B] tensors are contiguous 16.75 MB regions.  Each core computes
a partial [B] loss vector summed over its 8 towns; the host adds the 8
partials and takes the mean.

V5 dataflow (per core; HBM-bound, ~33.6 MB window data + 4.2 MB frames):
  - 16 chunk DMAs of [t4, h32, w4, b1024] (2.1 MB, 4 KB contiguous rows)
    ride the Sync HWDGE queue back-to-back (w-halves A/B per group)
  - frames are cast-DMAed to bf16 by GpSimd SWDGE (ofd/ifd per th-group);
    target/containment arrive host-prepacked on the Scalar HWDGE queue
  - per (side, th, bh) group, DVE tree: L1a/L1b fold w4->w2 per half
    (fp32 in -> bf16 out, 1x), L2 = A'+B' (bf16, 2x), L3 fold w2->w1
    (bf16, 2x), + frame slice (bf16, 2x) -> a[128=(t4,h32), 1024] bf16
  - PE transposes a in 8 bf16 128x128 blocks into 2 PSUM banks (f32);
    DVE tensor_reduce min (max for the ow side) over h -> m1/m2
  - coefficients c1 = bel*(1-cont), c2 = (1-bel)*cont computed up front;
    tail: w1=c1*m1 + c2*m2, reduce over t -> zb[128, 16], z out on the
    Scalar queue.  z[p, c] = partial loss for b = c*128 + p.
"""

import numpy as np

T, H, W, B = 64, 32, 8, 2048
NCORES = 8
TL = T // NCORES          # 8 local towns per core
NBC = B // 128            # 16 batch chunks of 128
BH = B // 2               # 1024: b-extent of one chunk

_CACHE = {}


def _build_program():
    import concourse.bass as bass
    import concourse.tile as tile
    from concourse import bacc, mybir

    f32 = mybir.dt.float32
    bf16 = mybir.dt.bfloat16
    u8 = mybir.dt.uint8
    AX = mybir.AxisListType
    OP = mybir.AluOpType

    nc = bacc.Bacc()
    iw = nc.declare_dram_parameter("iw", [TL, H, W, B], f32, isOutput=False)
    ow = nc.declare_dram_parameter("ow", [TL, H, W, B], f32, isOutput=False)
    # frames are pre-cast to bf16 on the host (they are 1.3% of the input
    # bytes; the on-chip tree is bf16 anyway) so they arrive early and cheap
    ofd = nc.declare_dram_parameter("ofd", [TL, H, B], bf16, isOutput=False)
    ifd = nc.declare_dram_parameter("ifd", [TL, H, B], bf16, isOutput=False)
    # host-prepacked: tgtp[p, c, t] = target[c*128+p, town t]; contp likewise
    tgtp = nc.declare_dram_parameter("tgtp", [128, NBC, TL], u8, isOutput=False)
    contp = nc.declare_dram_parameter("contp", [128, NBC, TL], f32, isOutput=False)
    z = nc.declare_dram_parameter("z", [128, NBC], f32, isOutput=True)

    ident = nc.inline_tensor(np.eye(128, dtype=np.float32), name="ident128")

    with tile.TileContext(nc) as tc:
        with (
            tc.tile_pool(name="const", bufs=1) as const_pool,
            tc.tile_pool(name="big", bufs=8) as big_pool,
            tc.tile_pool(name="frame", bufs=4) as frame_pool,
            tc.tile_pool(name="l1", bufs=3) as l1_pool,
            tc.tile_pool(name="l2", bufs=2) as l2_pool,
            tc.tile_pool(name="atile", bufs=2) as a_pool,
            tc.tile_pool(name="mres", bufs=1) as m_pool,
            tc.tile_pool(name="fin", bufs=1) as fin_pool,
            tc.tile_pool(name="ps", bufs=8, space="PSUM") as psum_pool,
        ):
            # ---- frame DMAs (bf16, host-cast) on the Scalar HWDGE queue ----
            frs = {}
            for side in range(2):
                src3 = ofd if side == 0 else ifd
                for th in range(2):
                    t0 = th * 4
                    fr = frame_pool.tile([128, B], bf16, tag="fr")
                    nc.scalar.dma_start(
                        fr[:],
                        src3[t0 : t0 + 4, :, :].rearrange("t h b -> (t h) b"),
                    )
                    frs[(side, th)] = fr

            # ---- small DMAs on the Scalar HWDGE queue ----
            tgt8 = fin_pool.tile([128, NBC * TL], u8, tag="tgt8")
            nc.scalar.dma_start(
                tgt8[:].rearrange("p (c t) -> p c t", t=TL), tgtp[:, :, :]
            )
            cT = fin_pool.tile([128, NBC * TL], f32, tag="cT")
            nc.scalar.dma_start(
                cT[:].rearrange("p (c t) -> p c t", t=TL), contp[:, :, :]
            )
            identf = const_pool.tile([128, 128], f32)
            nc.scalar.dma_start(identf[:], ident[:, :])
            identc = const_pool.tile([128, 128], bf16)
            nc.vector.tensor_copy(identc[:], identf[:])

            # ---- coefficients, computed up front on DVE ----
            bel = fin_pool.tile([128, NBC * TL], f32, tag="bel")
            nc.vector.tensor_copy(bel[:], tgt8[:])
            bc_t = fin_pool.tile([128, NBC * TL], f32, tag="bct")
            nc.vector.tensor_mul(bc_t[:], bel[:], cT[:])
            c1 = fin_pool.tile([128, NBC * TL], f32, tag="c1")
            nc.vector.tensor_sub(c1[:], bel[:], bc_t[:])
            c2 = fin_pool.tile([128, NBC * TL], f32, tag="c2")
            nc.vector.tensor_sub(c2[:], cT[:], bc_t[:])

            # m1/m2: col = bc*TL + t
            m1 = m_pool.tile([128, NBC * TL], f32, tag="m1")
            m2 = m_pool.tile([128, NBC * TL], f32, tag="m2")

            for side in range(2):
                src4 = iw if side == 0 else ow
                mdst = m1 if side == 0 else m2
                red_op = OP.min if side == 0 else OP.max
                mview = mdst[:].rearrange("p (c t) -> p c t", t=TL)

                for th in range(2):
                    t0 = th * 4
                    fr = frs[(side, th)]
                    for bh in range(2):
                        b0 = bh * BH
                        # two w-half chunks [128=(t4,h32), (w4 b1024)],
                        # one per HWDGE queue (Sync / Scalar) in parallel
                        bts = []
                        for wh in range(2):
                            bt = big_pool.tile([128, 4 * BH], f32, tag="big")
                            eng = nc.sync if wh == 0 else nc.scalar
                            eng.dma_start(
                                bt[:].rearrange("p (w b) -> p w b", w=4),
                                src4[
                                    t0 : t0 + 4, :, 4 * wh : 4 * wh + 4,
                                    b0 : b0 + BH,
                                ].rearrange("t h w b -> (t h) w b"),
                            )
                            bts.append(bt)

                        # L1 folds (fp32 in, bf16 out): DVE takes half A,
                        # GpSimd half B; then DVE bf16 2x for the rest
                        l1a = l1_pool.tile([128, 2 * BH], bf16, tag="l1")
                        nc.vector.tensor_add(
                            l1a[:], bts[0][:, 0 : 2 * BH], bts[0][:, 2 * BH : 4 * BH]
                        )
                        l1b = l1_pool.tile([128, 2 * BH], bf16, tag="l1")
                        nc.gpsimd.tensor_add(
                            l1b[:], bts[1][:, 0 : 2 * BH], bts[1][:, 2 * BH : 4 * BH]
                        )
                        l2t = l2_pool.tile([128, 2 * BH], bf16, tag="l2")
                        nc.vector.tensor_add(l2t[:], l1a[:], l1b[:])
                        at = a_pool.tile([128, BH], bf16, tag="a")
                        nc.vector.tensor_add(
                            at[:], l2t[:, 0:BH], l2t[:, BH : 2 * BH]
                        )
                        nc.vector.tensor_add(at[:], at[:], fr[:, b0 : b0 + BH])

                        # PE transposes: 4 bf16 128x128 blocks per PSUM bank
                        for g in range(2):
                            pt = psum_pool.tile([128, 512], bf16, tag="pt")
                            for q in range(4):
                                lc = g * 4 + q
                                nc.tensor.transpose(
                                    pt[:, q * 128 : (q + 1) * 128],
                                    at[:, lc * 128 : (lc + 1) * 128],
                                    identc[:],
                                )
                            gg = bh * 2 + g
                            nc.vector.tensor_reduce(
                                mview[:, gg * 4 : (gg + 1) * 4, t0 : t0 + 4],
                                pt[:].rearrange(
                                    "p (c t h) -> p c t h", t=4, h=H
                                ),
                                axis=AX.X,
                                op=red_op,
                            )

            # ---- final combine ----
            w1 = fin_pool.tile([128, NBC * TL], f32, tag="w1")
            nc.vector.tensor_mul(w1[:], c1[:], m1[:])
            w2 = fin_pool.tile([128, NBC * TL], f32, tag="w2")
            nc.vector.tensor_mul(w2[:], c2[:], m2[:])
            wt = fin_pool.tile([128, NBC * TL], f32, tag="wt")
            nc.vector.tensor_add(wt[:], w1[:], w2[:])

            zb = fin_pool.tile([128, NBC], f32, tag="zb")
            nc.vector.tensor_reduce(
                zb[:],
                wt[:].rearrange("p (c t) -> p c t", t=TL),
                axis=AX.X,
                op=OP.add,
            )
            nc.scalar.dma_start(z[:, :], zb[:])

    nc.finalize()
    return nc


def _get_program():
    if "nc" not in _CACHE:
        _CACHE["nc"] = _build_program()
    return _CACHE["nc"]


def _pack_small(arr2d: np.ndarray) -> np.ndarray:
    """[B, TL] -> [128, NBC, TL] with out[p, c, t] = arr2d[c*128+p, t]."""
    return np.ascontiguousarray(
        arr2d.reshape(NBC, 128, TL).transpose(1, 0, 2)
    )


def make_in_maps(
    inner_window_distances: np.ndarray,
    outer_window_distances: np.ndarray,
    outer_frame_distance: np.ndarray,
    inner_frame_distance: np.ndarray,
    containment: np.ndarray,
    target: np.ndarray,
) -> list[dict]:
    from ml_dtypes import bfloat16

    iw = np.ascontiguousarray(inner_window_distances, dtype=np.float32)
    owd = np.ascontiguousarray(outer_window_distances, dtype=np.float32)
    ofd = np.ascontiguousarray(outer_frame_distance, dtype=np.float32).astype(bfloat16)
    ifd = np.ascontiguousarray(inner_frame_distance, dtype=np.float32).astype(bfloat16)
    cont = np.ascontiguousarray(containment, dtype=np.float32)
    tgt = np.ascontiguousarray(target).view(np.uint8)

    in_maps = []
    for c in range(NCORES):
        t0, t1 = c * TL, (c + 1) * TL
        in_maps.append(
            {
                "iw": np.ascontiguousarray(iw[t0:t1]),
                "ow": np.ascontiguousarray(owd[t0:t1]),
                "ofd": np.ascontiguousarray(ofd[t0:t1]),
                "ifd": np.ascontiguousarray(ifd[t0:t1]),
                "tgtp": _pack_small(tgt[:, t0:t1]),
                "contp": _pack_small(np.ascontiguousarray(cont[t0:t1].T)),
            }
        )
    return in_maps


def kernel(
    inner_window_distances: np.ndarray,
    outer_window_distances: np.ndarray,
    outer_frame_distance: np.ndarray,
    inner_frame_distance: np.ndarray,
    containment: np.ndarray,
    target: np.ndarray,
) -> np.ndarray:
    from concourse.bass_utils import run_bass_kernel_spmd

    nc = _get_program()
    in_maps = make_in_maps(
        inner_window_distances,
        outer_window_distances,
        outer_frame_distance,
        inner_frame_distance,
        containment,
        target,
    )
    res = run_bass_kernel_spmd(nc, in_maps, list(range(NCORES)))

    # z[p, c] (per core) = partial loss for b = c*128 + p, summed over the
    # core's 8 towns.  Sum cores, flatten to [B], mean.
    acc = np.zeros((128, NBC), dtype=np.float64)
    for r in res.results:
        acc += r["z"].astype(np.float64)
    loss_b = acc.T.reshape(B)
    return np.float32(loss_b.mean())
